# revision 7
# baseline (speedup 1.0000x reference)
"""AttnBlock (GroupNorm + spatial self-attention + residual) on 8 TRN2 NeuronCores.

Data-parallel over batch: B=16 -> 2 batch elements per core. Each core runs the
full attention block for its 2 batch elements entirely on-chip; no collectives.

Layouts (per batch element, per core):
  x, r      : [c-part, n]   channel on partitions (4 tiles of 128), spatial free
  q, k      : [c-part, n]   from matmul(lhsT=wT, rhs=r)
  vT        : [n-part, c]   from matmul(lhsT=r, rhs=wT)  (transposed projection)
  scoresT   : [j-part, i]   = k^T q per (j-tile, i-chunk), accumulated over c
  E=exp(s)  : [j-part, i]   bf16, softmax numerator (logits are tiny: no max-sub)
  sums      : all-ones[128,128] @ E accumulates Sum_j E -> broadcast over partitions
  out       : [c-part, i]   = vT^T @ E accumulated over j-tiles, then * 1/sums
  proj, y   : [c-part, n]

Bias folding: bq/bk added at PSUM evac (per-partition bias). bv/bp folded on the
host into pb = wp@bv + bp (valid since sum_j softmax = 1), added at proj evac.
"""

import numpy as np
import ml_dtypes

import concourse.bass as bass
import concourse.bacc as bacc
import concourse.mybir as mybir
import concourse.tile as tile
from concourse.bass_utils import run_bass_kernel_spmd

B, C, HH, WW = 16, 512, 32, 32
N = HH * WW            # 1024 spatial positions
G = 32                 # groupnorm groups
GS = C // G            # 16 channels per group
EPS = 1e-6
P = 128
CT = C // P            # 4 channel tiles
NT = N // P            # 8 spatial tiles
CH = 512               # free-dim chunk (one PSUM bank of fp32)
NCH = N // CH          # 2 chunks
NCORES = 8
BPC = B // NCORES      # 2 batch elements per core
SCALE = float(int(C) ** -0.5)

F32 = mybir.dt.float32
BF16 = mybir.dt.bfloat16
AF = mybir.ActivationFunctionType


def _build_program() -> bass.Bass:
    nc = bacc.Bacc("TRN2", target_bir_lowering=False, num_devices=NCORES)

    x_in = nc.declare_dram_parameter("x_in", [BPC, C, N], F32, isOutput=False)
    w_in = {
        w: nc.declare_dram_parameter(w + "T", [C, C], BF16, isOutput=False)
        for w in ("wq", "wk", "wv", "wp")
    }
    # cols[:, 0]=gn_w, 1=gn_b, 2=bq, 3=bk, 4=pb   (per-partition packing, [P, 5, CT])
    cols_in = nc.declare_dram_parameter("cols", [P, 5, CT], F32, isOutput=False)
    mmat_in = nc.declare_dram_parameter("mmat", [P, P], F32, isOutput=False)
    ones_in = nc.declare_dram_parameter("onesm", [P, P], BF16, isOutput=False)
    y_out = nc.declare_dram_parameter("y_out", [BPC, C, N], F32, isOutput=True)

    with tile.TileContext(nc) as tc:
        with (
            tc.tile_pool(name="const", bufs=1) as const,
            tc.tile_pool(name="act", bufs=1) as act,
            tc.tile_pool(name="small", bufs=2) as small,
            tc.tile_pool(name="psum", bufs=1, space="PSUM") as psum,
        ):
            w_sb = {}
            for w in ("wq", "wk", "wv", "wp"):
                wt = const.tile([P, CT, C], BF16, name=f"{w}_sb", tag=f"{w}_sb")
                nc.gpsimd.dma_start(out=wt, in_=w_in[w].rearrange("(t p) o -> p t o", p=P))
                w_sb[w] = wt
            cols = const.tile([P, 5, CT], F32, name="cols_sb", tag="cols_sb")
            nc.gpsimd.dma_start(out=cols, in_=cols_in[:, :, :])
            mmat = const.tile([P, P], F32, name="mmat_sb", tag="mmat_sb")
            nc.gpsimd.dma_start(out=mmat, in_=mmat_in[:, :])
            onesm = const.tile([P, P], BF16, name="ones_sb", tag="ones_sb")
            nc.gpsimd.dma_start(out=onesm, in_=ones_in[:, :])
            eps_sb = const.tile([P, 1], F32, name="eps_sb", tag="eps_sb")
            nc.vector.memset(eps_sb, EPS)

            for b in range(BPC):
                # ---------- load x ----------
                x_t = act.tile([P, CT, N], F32, name="x_t", tag="x", bufs=2)
                nc.sync.dma_start(out=x_t, in_=x_in[b].rearrange("(t p) n -> p t n", p=P))

                # ---------- GroupNorm stats ----------
                bn6 = small.tile([P, CT, 2, 6], F32, name="bn6", tag="bn6")
                mv = small.tile([P, CT, 2], F32, name="mv", tag="mv")
                for ct in range(CT):
                    for h in range(2):
                        nc.vector.bn_stats(
                            out=bn6[:, ct, h, :], in_=x_t[:, ct, h * CH:(h + 1) * CH]
                        )
                    nc.vector.bn_aggr(out=mv[:, ct, :], in_=bn6[:, ct, :, :])
                # mv[:,:,0]=mean_c, mv[:,:,1]=var_c  ->  (mean_c, E[x^2]_c) in place
                msq = small.tile([P, CT], F32, name="msq", tag="msq")
                nc.vector.tensor_mul(msq, mv[:, :, 0], mv[:, :, 0])
                nc.vector.tensor_add(mv[:, :, 1], mv[:, :, 1], msq)

                # group-sum across the 16-channel blocks via block-diagonal matmul
                gn_ps = psum.tile([P, CT, 2], F32, name="gn_ps", tag="mm", bufs=2)
                for ct in range(CT):
                    nc.tensor.matmul(
                        gn_ps[:, ct, :], lhsT=mmat, rhs=mv[:, ct, :],
                        start=True, stop=True,
                    )
                # per-channel group mean / E[x^2]
                gsc = small.tile([P, CT, 2], F32, name="gsc", tag="gsc")
                nc.scalar.mul(gsc, gn_ps, 1.0 / GS)
                var_t = small.tile([P, CT], F32, name="var_t", tag="var_t")
                nc.vector.tensor_mul(msq, gsc[:, :, 0], gsc[:, :, 0])
                nc.vector.tensor_sub(var_t, gsc[:, :, 1], msq)
                nc.scalar.activation(var_t, var_t, AF.Sqrt, bias=eps_sb)
                nc.vector.reciprocal(var_t, var_t)          # rstd
                scl = small.tile([P, CT], F32, name="scl", tag="scl")
                sh_t = small.tile([P, CT], F32, name="sh_t", tag="sh_t")
                nc.vector.tensor_mul(scl, cols[:, 0, :], var_t)   # gn_w * rstd
                nc.vector.tensor_mul(sh_t, gsc[:, :, 0], scl)     # mean * scale
                nc.vector.tensor_sub(sh_t, cols[:, 1, :], sh_t)   # gn_b - mean*scale

                r_bf = act.tile([P, CT, N], BF16, name="r_bf", tag="r", bufs=2)
                for ct in range(CT):
                    nc.scalar.activation(
                        r_bf[:, ct, :], x_t[:, ct, :], AF.Identity,
                        scale=scl[:, ct:ct + 1], bias=sh_t[:, ct:ct + 1],
                    )

                # ---------- q/k (c-part layout) and vT (n-part layout) ----------
                q_bf = act.tile([P, CT, N], BF16, name="q_bf", tag="q", bufs=1)
                k_bf = act.tile([P, CT, N], BF16, name="k_bf", tag="k", bufs=1)
                vT_bf = act.tile([P, NT, C], BF16, name="vT_bf", tag="v", bufs=1)
                for ot in range(CT):
                    for chn in range(NCH):
                        nsl = slice(chn * CH, (chn + 1) * CH)
                        for nm, wname, cidx, dst in (
                            ("q", "wq", 2, q_bf), ("k", "wk", 3, k_bf)
                        ):
                            ps = psum.tile([P, CH], F32, name=f"{nm}_ps", tag="mm", bufs=2)
                            for ct in range(CT):
                                nc.tensor.matmul(
                                    ps,
                                    lhsT=w_sb[wname][:, ct, ot * P:(ot + 1) * P],
                                    rhs=r_bf[:, ct, nsl],
                                    start=(ct == 0), stop=(ct == CT - 1),
                                )
                            nc.scalar.activation(
                                dst[:, ot, nsl], ps, AF.Identity,
                                bias=cols[:, cidx, ot:ot + 1],
                            )
                for nt in range(NT):
                    ps = psum.tile([P, CH], F32, name="v_ps", tag="mm", bufs=2)
                    for ct in range(CT):
                        nc.tensor.matmul(
                            ps,
                            lhsT=r_bf[:, ct, nt * P:(nt + 1) * P],
                            rhs=w_sb["wv"][:, ct, :],
                            start=(ct == 0), stop=(ct == CT - 1),
                        )
                    nc.scalar.copy(vT_bf[:, nt, :], ps)

                # ---------- attention ----------
                outn_bf = act.tile([P, CT, N], BF16, name="outn_bf", tag="outn", bufs=1)
                for chn in range(NCH):
                    isl = slice(chn * CH, (chn + 1) * CH)
                    att_ps = psum.tile([P, CT, CH], F32, name="att_ps", tag="att", bufs=1)
                    sums_ps = psum.tile([P, CH], F32, name="sums_ps", tag="mm", bufs=2)
                    for jt in range(NT):
                        s_ps = psum.tile([P, CH], F32, name="s_ps", tag="scores", bufs=2)
                        for ct in range(CT):
                            nc.tensor.matmul(
                                s_ps,
                                lhsT=k_bf[:, ct, jt * P:(jt + 1) * P],
                                rhs=q_bf[:, ct, isl],
                                start=(ct == 0), stop=(ct == CT - 1),
                            )
                        e_bf = small.tile([P, CH], BF16, name="e_bf", tag="E", bufs=4)
                        nc.scalar.activation(e_bf, s_ps, AF.Exp, scale=SCALE)
                        for ct in range(CT):
                            nc.tensor.matmul(
                                att_ps[:, ct, :],
                                lhsT=vT_bf[:, jt, ct * P:(ct + 1) * P],
                                rhs=e_bf,
                                start=(jt == 0), stop=(jt == NT - 1),
                            )
                        nc.tensor.matmul(
                            sums_ps, lhsT=onesm, rhs=e_bf,
                            start=(jt == 0), stop=(jt == NT - 1),
                        )
                    recip = small.tile([P, CH], F32, name="recip", tag="recip", bufs=2)
                    nc.vector.reciprocal(recip, sums_ps)
                    for ct in range(CT):
                        nc.vector.tensor_mul(
                            outn_bf[:, ct, isl], att_ps[:, ct, :], recip
                        )

                # ---------- proj + residual ----------
                y_t = act.tile([P, CT, N], F32, name="y_t", tag="y", bufs=2)
                for ot in range(CT):
                    for chn in range(NCH):
                        nsl = slice(chn * CH, (chn + 1) * CH)
                        ps = psum.tile([P, CH], F32, name="p_ps", tag="mm", bufs=2)
                        for ct in range(CT):
                            nc.tensor.matmul(
                                ps,
                                lhsT=w_sb["wp"][:, ct, ot * P:(ot + 1) * P],
                                rhs=outn_bf[:, ct, nsl],
                                start=(ct == 0), stop=(ct == CT - 1),
                            )
                        pr = small.tile([P, CH], F32, name="pr", tag="pr", bufs=3)
                        nc.scalar.activation(
                            pr, ps, AF.Identity, bias=cols[:, 4, ot:ot + 1]
                        )
                        nc.vector.tensor_add(
                            y_t[:, ot, nsl], pr, x_t[:, ot, nsl]
                        )
                nc.sync.dma_start(
                    out=y_out[b].rearrange("(t p) n -> p t n", p=P), in_=y_t
                )
    nc.compile()
    return nc


def _prep_in_maps(inputs) -> list[dict]:
    f32 = np.float32
    x = np.asarray(inputs["x"], f32).reshape(B, C, N)

    def t_bf(w):
        return np.ascontiguousarray(np.asarray(w, f32).T).astype(ml_dtypes.bfloat16)

    def packc(v):
        return np.ascontiguousarray(np.asarray(v, f32).reshape(CT, P).T)

    pb = (
        np.asarray(inputs["wp"], f32) @ np.asarray(inputs["bv"], f32)
        + np.asarray(inputs["bp"], f32)
    )
    cols = np.ascontiguousarray(
        np.stack(
            [
                packc(inputs["gn_w"]), packc(inputs["gn_b"]),
                packc(inputs["bq"]), packc(inputs["bk"]), packc(pb),
            ],
            axis=1,
        )
    )  # [P, 5, CT]
    mmat = np.kron(
        np.eye(P // GS, dtype=f32), np.ones((GS, GS), f32)
    )  # [128,128] block-diagonal group-sum matrix
    onesm = np.ones((P, P), ml_dtypes.bfloat16)
    shared = dict(
        wqT=t_bf(inputs["wq"]), wkT=t_bf(inputs["wk"]),
        wvT=t_bf(inputs["wv"]), wpT=t_bf(inputs["wp"]),
        cols=cols, mmat=mmat, onesm=onesm,
    )
    return [
        dict(x_in=np.ascontiguousarray(x[c * BPC:(c + 1) * BPC]), **shared)
        for c in range(NCORES)
    ]


_PROG = None


def _run(inputs, **spmd_kwargs):
    global _PROG
    if _PROG is None:
        _PROG = _build_program()
    in_maps = _prep_in_maps(inputs)
    res = run_bass_kernel_spmd(_PROG, in_maps, list(range(NCORES)), **spmd_kwargs)
    y = np.concatenate(
        [np.asarray(res.results[i]["y_out"], np.float32) for i in range(NCORES)],
        axis=0,
    ).reshape(B, C, HH, WW)
    return y, res


def kernel(**inputs) -> np.ndarray:
    y, _ = _run(inputs)
    return y


# revision 8
# speedup vs baseline: 12.0348x; 12.0348x over previous
"""AttnBlock (GroupNorm + spatial self-attention + residual) on 8 TRN2 NeuronCores.

Data-parallel over batch: B=16 -> 2 batch elements per core. Each core runs the
full attention block for its 2 batch elements entirely on-chip; no collectives.

Layouts (per batch element, per core):
  x, r      : [c-part, n]   channel on partitions (4 tiles of 128), spatial free
  q, k      : [c-part, n]   from matmul(lhsT=wT, rhs=r)
  vT        : [n-part, c]   from matmul(lhsT=r, rhs=wT)  (transposed projection)
  scoresT   : [j-part, i]   = k^T q per (j-tile, i-chunk), accumulated over c
  E=exp(s)  : [j-part, i]   bf16, softmax numerator (logits are tiny: no max-sub)
  sums      : all-ones[128,128] @ E accumulates Sum_j E -> broadcast over partitions
  out       : [c-part, i]   = vT^T @ E accumulated over j-tiles, then * 1/sums
  proj, y   : [c-part, n]

Bias folding: bq/bk added at PSUM evac (per-partition bias). bv/bp folded on the
host into pb = wp@bv + bp (valid since sum_j softmax = 1), added at proj evac.
"""

import numpy as np
import ml_dtypes

import concourse.bass as bass
import concourse.bacc as bacc
import concourse.mybir as mybir
import concourse.tile as tile
from concourse.bass_utils import run_bass_kernel_spmd

B, C, HH, WW = 16, 512, 32, 32
N = HH * WW            # 1024 spatial positions
G = 32                 # groupnorm groups
GS = C // G            # 16 channels per group
EPS = 1e-6
P = 128
CT = C // P            # 4 channel tiles
NT = N // P            # 8 spatial tiles
CH = 512               # free-dim chunk (one PSUM bank of fp32)
NCH = N // CH          # 2 chunks
NCORES = 8
BPC = B // NCORES      # 2 batch elements per core
SCALE = float(int(C) ** -0.5)

F32 = mybir.dt.float32
BF16 = mybir.dt.bfloat16
AF = mybir.ActivationFunctionType


def _build_program(loop_reps: int = 1) -> bass.Bass:
    nc = bacc.Bacc("TRN2", target_bir_lowering=False, num_devices=NCORES)

    x_in = nc.declare_dram_parameter("x_in", [BPC, C, N], F32, isOutput=False)
    w_in = {
        w: nc.declare_dram_parameter(w + "T", [C, C], BF16, isOutput=False)
        for w in ("wq", "wk", "wv", "wp")
    }
    # cols[:, 0]=gn_w, 1=gn_b, 2=bq, 3=bk, 4=pb   (per-partition packing, [P, 5, CT])
    cols_in = nc.declare_dram_parameter("cols", [P, 5, CT], F32, isOutput=False)
    mmat_in = nc.declare_dram_parameter("mmat", [P, P], F32, isOutput=False)
    ones_in = nc.declare_dram_parameter("onesm", [P, P], BF16, isOutput=False)
    y_out = nc.declare_dram_parameter("y_out", [BPC, C, N], F32, isOutput=True)

    with tile.TileContext(nc) as tc:
        with (
            tc.tile_pool(name="const", bufs=1) as const,
            tc.tile_pool(name="act", bufs=1) as act,
            tc.tile_pool(name="small", bufs=2) as small,
            tc.tile_pool(name="psum", bufs=1, space="PSUM") as psum,
        ):
            w_sb = {}
            for w in ("wq", "wk", "wv", "wp"):
                wt = const.tile([P, CT, C], BF16, name=f"{w}_sb", tag=f"{w}_sb")
                nc.gpsimd.dma_start(out=wt, in_=w_in[w].rearrange("(t p) o -> p t o", p=P))
                w_sb[w] = wt
            cols = const.tile([P, 5, CT], F32, name="cols_sb", tag="cols_sb")
            nc.gpsimd.dma_start(out=cols, in_=cols_in[:, :, :])
            mmat = const.tile([P, P], F32, name="mmat_sb", tag="mmat_sb")
            nc.gpsimd.dma_start(out=mmat, in_=mmat_in[:, :])
            onesm = const.tile([P, P], BF16, name="ones_sb", tag="ones_sb")
            nc.gpsimd.dma_start(out=onesm, in_=ones_in[:, :])
            eps_sb = const.tile([P, 1], F32, name="eps_sb", tag="eps_sb")
            nc.vector.memset(eps_sb, EPS)

            import contextlib
            loop_cm = (
                tc.For_i(0, loop_reps, 1) if loop_reps > 1
                else contextlib.nullcontext()
            )
            with loop_cm:
                _emit_body(nc, tc, act, small, psum, x_in, y_out, w_sb, cols,
                           mmat, onesm, eps_sb)
    nc.compile()
    return nc


def _emit_body(nc, tc, act, small, psum, x_in, y_out, w_sb, cols, mmat, onesm,
               eps_sb):
    if True:
        if True:
            for b in range(BPC):
                # ---------- load x ----------
                x_t = act.tile([P, CT, N], F32, name="x_t", tag="x", bufs=2)
                nc.sync.dma_start(out=x_t, in_=x_in[b].rearrange("(t p) n -> p t n", p=P))

                # ---------- GroupNorm stats ----------
                bn6 = small.tile([P, CT, 2, 6], F32, name="bn6", tag="bn6")
                mv = small.tile([P, CT, 2], F32, name="mv", tag="mv")
                for ct in range(CT):
                    for h in range(2):
                        nc.vector.bn_stats(
                            out=bn6[:, ct, h, :], in_=x_t[:, ct, h * CH:(h + 1) * CH]
                        )
                    nc.vector.bn_aggr(out=mv[:, ct, :], in_=bn6[:, ct, :, :])
                # mv[:,:,0]=mean_c, mv[:,:,1]=var_c  ->  (mean_c, E[x^2]_c) in place
                msq = small.tile([P, CT], F32, name="msq", tag="msq")
                nc.vector.tensor_mul(msq, mv[:, :, 0], mv[:, :, 0])
                nc.vector.tensor_add(mv[:, :, 1], mv[:, :, 1], msq)

                # group-sum across the 16-channel blocks via block-diagonal matmul
                gn_ps = psum.tile([P, CT, 2], F32, name="gn_ps", tag="mm", bufs=2)
                for ct in range(CT):
                    nc.tensor.matmul(
                        gn_ps[:, ct, :], lhsT=mmat, rhs=mv[:, ct, :],
                        start=True, stop=True,
                    )
                # per-channel group mean / E[x^2]
                gsc = small.tile([P, CT, 2], F32, name="gsc", tag="gsc")
                nc.scalar.mul(gsc, gn_ps, 1.0 / GS)
                var_t = small.tile([P, CT], F32, name="var_t", tag="var_t")
                nc.vector.tensor_mul(msq, gsc[:, :, 0], gsc[:, :, 0])
                nc.vector.tensor_sub(var_t, gsc[:, :, 1], msq)
                nc.scalar.activation(var_t, var_t, AF.Sqrt, bias=eps_sb)
                nc.vector.reciprocal(var_t, var_t)          # rstd
                scl = small.tile([P, CT], F32, name="scl", tag="scl")
                sh_t = small.tile([P, CT], F32, name="sh_t", tag="sh_t")
                nc.vector.tensor_mul(scl, cols[:, 0, :], var_t)   # gn_w * rstd
                nc.vector.tensor_mul(sh_t, gsc[:, :, 0], scl)     # mean * scale
                nc.vector.tensor_sub(sh_t, cols[:, 1, :], sh_t)   # gn_b - mean*scale

                r_bf = act.tile([P, CT, N], BF16, name="r_bf", tag="r", bufs=2)
                for ct in range(CT):
                    nc.scalar.activation(
                        r_bf[:, ct, :], x_t[:, ct, :], AF.Identity,
                        scale=scl[:, ct:ct + 1], bias=sh_t[:, ct:ct + 1],
                    )

                # ---------- q/k (c-part layout) and vT (n-part layout) ----------
                q_bf = act.tile([P, CT, N], BF16, name="q_bf", tag="q", bufs=1)
                k_bf = act.tile([P, CT, N], BF16, name="k_bf", tag="k", bufs=1)
                vT_bf = act.tile([P, NT, C], BF16, name="vT_bf", tag="v", bufs=1)
                for ot in range(CT):
                    for chn in range(NCH):
                        nsl = slice(chn * CH, (chn + 1) * CH)
                        for nm, wname, cidx, dst in (
                            ("q", "wq", 2, q_bf), ("k", "wk", 3, k_bf)
                        ):
                            ps = psum.tile([P, CH], F32, name=f"{nm}_ps", tag="mm", bufs=2)
                            for ct in range(CT):
                                nc.tensor.matmul(
                                    ps,
                                    lhsT=w_sb[wname][:, ct, ot * P:(ot + 1) * P],
                                    rhs=r_bf[:, ct, nsl],
                                    start=(ct == 0), stop=(ct == CT - 1),
                                )
                            nc.scalar.activation(
                                dst[:, ot, nsl], ps, AF.Identity,
                                bias=cols[:, cidx, ot:ot + 1],
                            )
                for nt in range(NT):
                    ps = psum.tile([P, CH], F32, name="v_ps", tag="mm", bufs=2)
                    for ct in range(CT):
                        nc.tensor.matmul(
                            ps,
                            lhsT=r_bf[:, ct, nt * P:(nt + 1) * P],
                            rhs=w_sb["wv"][:, ct, :],
                            start=(ct == 0), stop=(ct == CT - 1),
                        )
                    nc.scalar.copy(vT_bf[:, nt, :], ps)

                # ---------- attention ----------
                outn_bf = act.tile([P, CT, N], BF16, name="outn_bf", tag="outn", bufs=1)
                for chn in range(NCH):
                    isl = slice(chn * CH, (chn + 1) * CH)
                    att_ps = psum.tile([P, CT, CH], F32, name="att_ps", tag="att", bufs=1)
                    sums_ps = psum.tile([P, CH], F32, name="sums_ps", tag="mm", bufs=2)
                    for jt in range(NT):
                        s_ps = psum.tile([P, CH], F32, name="s_ps", tag="scores", bufs=2)
                        for ct in range(CT):
                            nc.tensor.matmul(
                                s_ps,
                                lhsT=k_bf[:, ct, jt * P:(jt + 1) * P],
                                rhs=q_bf[:, ct, isl],
                                start=(ct == 0), stop=(ct == CT - 1),
                            )
                        e_bf = small.tile([P, CH], BF16, name="e_bf", tag="E", bufs=4)
                        nc.scalar.activation(e_bf, s_ps, AF.Exp, scale=SCALE)
                        for ct in range(CT):
                            nc.tensor.matmul(
                                att_ps[:, ct, :],
                                lhsT=vT_bf[:, jt, ct * P:(ct + 1) * P],
                                rhs=e_bf,
                                start=(jt == 0), stop=(jt == NT - 1),
                            )
                        nc.tensor.matmul(
                            sums_ps, lhsT=onesm, rhs=e_bf,
                            start=(jt == 0), stop=(jt == NT - 1),
                        )
                    recip = small.tile([P, CH], F32, name="recip", tag="recip", bufs=2)
                    nc.vector.reciprocal(recip, sums_ps)
                    for ct in range(CT):
                        nc.vector.tensor_mul(
                            outn_bf[:, ct, isl], att_ps[:, ct, :], recip
                        )

                # ---------- proj + residual ----------
                y_t = act.tile([P, CT, N], F32, name="y_t", tag="y", bufs=2)
                for ot in range(CT):
                    for chn in range(NCH):
                        nsl = slice(chn * CH, (chn + 1) * CH)
                        ps = psum.tile([P, CH], F32, name="p_ps", tag="mm", bufs=2)
                        for ct in range(CT):
                            nc.tensor.matmul(
                                ps,
                                lhsT=w_sb["wp"][:, ct, ot * P:(ot + 1) * P],
                                rhs=outn_bf[:, ct, nsl],
                                start=(ct == 0), stop=(ct == CT - 1),
                            )
                        pr = small.tile([P, CH], F32, name="pr", tag="pr", bufs=3)
                        nc.scalar.activation(
                            pr, ps, AF.Identity, bias=cols[:, 4, ot:ot + 1]
                        )
                        nc.vector.tensor_add(
                            y_t[:, ot, nsl], pr, x_t[:, ot, nsl]
                        )
                nc.sync.dma_start(
                    out=y_out[b].rearrange("(t p) n -> p t n", p=P), in_=y_t
                )


def _prep_in_maps(inputs) -> list[dict]:
    f32 = np.float32
    x = np.asarray(inputs["x"], f32).reshape(B, C, N)

    def t_bf(w):
        return np.ascontiguousarray(np.asarray(w, f32).T).astype(ml_dtypes.bfloat16)

    def packc(v):
        return np.ascontiguousarray(np.asarray(v, f32).reshape(CT, P).T)

    pb = (
        np.asarray(inputs["wp"], f32) @ np.asarray(inputs["bv"], f32)
        + np.asarray(inputs["bp"], f32)
    )
    cols = np.ascontiguousarray(
        np.stack(
            [
                packc(inputs["gn_w"]), packc(inputs["gn_b"]),
                packc(inputs["bq"]), packc(inputs["bk"]), packc(pb),
            ],
            axis=1,
        )
    )  # [P, 5, CT]
    mmat = np.kron(
        np.eye(P // GS, dtype=f32), np.ones((GS, GS), f32)
    )  # [128,128] block-diagonal group-sum matrix
    onesm = np.ones((P, P), ml_dtypes.bfloat16)
    shared = dict(
        wqT=t_bf(inputs["wq"]), wkT=t_bf(inputs["wk"]),
        wvT=t_bf(inputs["wv"]), wpT=t_bf(inputs["wp"]),
        cols=cols, mmat=mmat, onesm=onesm,
    )
    return [
        dict(x_in=np.ascontiguousarray(x[c * BPC:(c + 1) * BPC]), **shared)
        for c in range(NCORES)
    ]


_PROG = None


def _run(inputs, **spmd_kwargs):
    global _PROG
    if _PROG is None:
        _PROG = _build_program()
    in_maps = _prep_in_maps(inputs)
    res = run_bass_kernel_spmd(_PROG, in_maps, list(range(NCORES)), **spmd_kwargs)
    y = np.concatenate(
        [np.asarray(res.results[i]["y_out"], np.float32) for i in range(NCORES)],
        axis=0,
    ).reshape(B, C, HH, WW)
    return y, res


def kernel(**inputs) -> np.ndarray:
    y, _ = _run(inputs)
    return y


# revision 12
# speedup vs baseline: 14.2160x; 1.1812x over previous
"""AttnBlock (GroupNorm + spatial self-attention + residual) on 8 TRN2 NeuronCores.

Data-parallel over batch: B=16 -> 2 batch elements per core. Each core runs the
full attention block for its 2 batch elements entirely on-chip; no collectives.

Layouts (per batch element, per core):
  x, r      : [c-part, n]   channel on partitions (4 tiles of 128), spatial free
  q, k      : [c-part, n]   from matmul(lhsT=wT, rhs=r)
  vT        : [n-part, c]   from matmul(lhsT=r, rhs=wT)  (transposed projection)
  scoresT   : [j-part, i]   = k^T q per (j-tile, i-chunk), accumulated over c
  E=exp(s)  : [j-part, i]   bf16, softmax numerator (logits are tiny: no max-sub)
  sums      : all-ones[128,128] @ E accumulates Sum_j E -> broadcast over partitions
  out       : [c-part, i]   = vT^T @ E accumulated over j-tiles, then * 1/sums
  proj, y   : [c-part, n]

Bias folding: bq/bk added at PSUM evac (per-partition bias). bv/bp folded on the
host into pb = wp@bv + bp (valid since sum_j softmax = 1), added at proj evac.
"""

import numpy as np
import ml_dtypes

import concourse.bass as bass
import concourse.bacc as bacc
import concourse.mybir as mybir
import concourse.tile as tile
from concourse.bass_utils import run_bass_kernel_spmd

B, C, HH, WW = 16, 512, 32, 32
N = HH * WW            # 1024 spatial positions
G = 32                 # groupnorm groups
GS = C // G            # 16 channels per group
EPS = 1e-6
P = 128
CT = C // P            # 4 channel tiles
NT = N // P            # 8 spatial tiles
CH = 512               # free-dim chunk (one PSUM bank of fp32)
NCH = N // CH          # 2 chunks
NCORES = 8
BPC = B // NCORES      # 2 batch elements per core
SCALE = float(int(C) ** -0.5)

F32 = mybir.dt.float32
BF16 = mybir.dt.bfloat16
FP8 = mybir.dt.float8e4
AF = mybir.ActivationFunctionType
ATT_FP8 = True          # fp8e4m3 + DoubleRow for scores/out/sums matmuls
DR = mybir.MatmulPerfMode.DoubleRow


def _build_program(loop_reps: int = 1) -> bass.Bass:
    nc = bacc.Bacc("TRN2", target_bir_lowering=False, num_devices=NCORES)

    x_in = nc.declare_dram_parameter("x_in", [BPC, C, N], F32, isOutput=False)
    w_in = {
        w: nc.declare_dram_parameter(w + "T", [C, C], BF16, isOutput=False)
        for w in ("wq", "wk", "wv", "wp")
    }
    # cols[:, 0]=gn_w, 1=gn_b, 2=bq, 3=bk, 4=pb   (per-partition packing, [P, 5, CT])
    cols_in = nc.declare_dram_parameter("cols", [P, 5, CT], F32, isOutput=False)
    mmat_in = nc.declare_dram_parameter("mmat", [P, P], F32, isOutput=False)
    ones_in = nc.declare_dram_parameter("onesm", [P, P], BF16, isOutput=False)
    y_out = nc.declare_dram_parameter("y_out", [BPC, C, N], F32, isOutput=True)

    with tile.TileContext(nc) as tc:
        with (
            tc.tile_pool(name="const", bufs=1) as const,
            tc.tile_pool(name="act", bufs=1) as act,
            tc.tile_pool(name="small", bufs=2) as small,
            tc.tile_pool(name="psum", bufs=1, space="PSUM") as psum,
        ):
            cols = const.tile([P, 5, CT], F32, name="cols_sb", tag="cols_sb")
            nc.gpsimd.dma_start(out=cols, in_=cols_in[:, :, :])
            mmat = const.tile([P, P], F32, name="mmat_sb", tag="mmat_sb")
            nc.gpsimd.dma_start(out=mmat, in_=mmat_in[:, :])
            onesm = const.tile([P, P], BF16, name="ones_sb", tag="ones_sb")
            nc.gpsimd.dma_start(out=onesm, in_=ones_in[:, :])
            ones8 = const.tile([P, 2, P], FP8, name="ones8_sb", tag="ones8_sb")
            nc.gpsimd.memset(ones8, 1.0)
            eps_sb = const.tile([P, 1], F32, name="eps_sb", tag="eps_sb")
            nc.vector.memset(eps_sb, EPS)
            w_sb = {}
            for w in ("wq", "wk", "wv", "wp"):
                wt = const.tile([P, CT, C], BF16, name=f"{w}_sb", tag=f"{w}_sb")
                nc.scalar.dma_start(out=wt, in_=w_in[w].rearrange("(t p) o -> p t o", p=P))
                w_sb[w] = wt

            import contextlib
            loop_cm = (
                tc.For_i(0, loop_reps, 1) if loop_reps > 1
                else contextlib.nullcontext()
            )
            with loop_cm:
                _emit_body(nc, tc, act, small, psum, x_in, y_out, w_sb, cols,
                           mmat, onesm, ones8, eps_sb)
    nc.compile()
    return nc


def _emit_body(nc, tc, act, small, psum, x_in, y_out, w_sb, cols, mmat, onesm,
               ones8, eps_sb):
    xs, rs = [], []
    # ---------- Phase 1: GroupNorm for both batches (per-c-tile pipeline) ----
    # Hoisted ahead of all projections so DVE/ACT compute batch b+1's GN while
    # PE runs batch b's matmuls, and so PE work starts after only one c-tile
    # of x has landed.
    for b in range(BPC):
        x_t = act.tile([P, CT, N], F32, name="x_t", tag="x", bufs=2)
        r_bf = act.tile([P, CT, N], BF16, name="r_bf", tag="r", bufs=2)
        bn6 = small.tile([P, CT, 2, 6], F32, name="bn6", tag="bn6")
        mv = small.tile([P, CT, 2], F32, name="mv", tag="mv")
        msq = small.tile([P, CT], F32, name="msq", tag="msq")
        gsc = small.tile([P, CT, 2], F32, name="gsc", tag="gsc")
        var_t = small.tile([P, CT], F32, name="var_t", tag="var_t")
        scl = small.tile([P, CT], F32, name="scl", tag="scl")
        sh_t = small.tile([P, CT], F32, name="sh_t", tag="sh_t")
        xr = x_in[b].rearrange("(t p) n -> p t n", p=P)
        for ct in range(CT):
            nc.sync.dma_start(out=x_t[:, ct, :], in_=xr[:, ct, :])
            for h in range(2):
                nc.vector.bn_stats(
                    out=bn6[:, ct, h, :], in_=x_t[:, ct, h * CH:(h + 1) * CH]
                )
            nc.vector.bn_aggr(out=mv[:, ct, :], in_=bn6[:, ct, :, :])
            # (mean, var) -> (mean, E[x^2]) in place
            nc.vector.tensor_mul(msq[:, ct:ct + 1], mv[:, ct, 0:1], mv[:, ct, 0:1])
            nc.vector.tensor_add(mv[:, ct, 1:2], mv[:, ct, 1:2], msq[:, ct:ct + 1])
            # group-sum across 16-channel blocks (block-diagonal matmul)
            gn_ps = psum.tile([P, 2], F32, name="gn_ps", tag="mm", bufs=2)
            nc.tensor.matmul(gn_ps, lhsT=mmat, rhs=mv[:, ct, :],
                             start=True, stop=True)
            nc.scalar.mul(gsc[:, ct, :], gn_ps, 1.0 / GS)
            nc.vector.tensor_mul(msq[:, ct:ct + 1], gsc[:, ct, 0:1], gsc[:, ct, 0:1])
            nc.vector.tensor_sub(var_t[:, ct:ct + 1], gsc[:, ct, 1:2], msq[:, ct:ct + 1])
            nc.scalar.activation(var_t[:, ct:ct + 1], var_t[:, ct:ct + 1],
                                 AF.Sqrt, bias=eps_sb)
            nc.vector.reciprocal(var_t[:, ct:ct + 1], var_t[:, ct:ct + 1])
            nc.vector.tensor_mul(scl[:, ct:ct + 1], cols[:, 0, ct:ct + 1],
                                 var_t[:, ct:ct + 1])
            nc.vector.tensor_mul(sh_t[:, ct:ct + 1], gsc[:, ct, 0:1],
                                 scl[:, ct:ct + 1])
            nc.vector.tensor_sub(sh_t[:, ct:ct + 1], cols[:, 1, ct:ct + 1],
                                 sh_t[:, ct:ct + 1])
            nc.scalar.activation(
                r_bf[:, ct, :], x_t[:, ct, :], AF.Identity,
                scale=scl[:, ct:ct + 1], bias=sh_t[:, ct:ct + 1],
            )
        xs.append(x_t)
        rs.append(r_bf)

    # ---------- Phase 2: per-batch projections + attention ----------
    for b in range(BPC):
        x_t, r_bf = xs[b], rs[b]
        ATT_DT = FP8 if ATT_FP8 else BF16
        q_bf = act.tile([P, CT, N], ATT_DT, name="q_bf", tag="q", bufs=1)
        k_bf = act.tile([P, CT, N], ATT_DT, name="k_bf", tag="k", bufs=1)
        vT_bf = act.tile([P, NT, C], ATT_DT, name="vT_bf", tag="v", bufs=1)
        for ot in range(CT):
            for chn in range(NCH):
                nsl = slice(chn * CH, (chn + 1) * CH)
                for nm, wname, cidx, dst in (
                    ("q", "wq", 2, q_bf), ("k", "wk", 3, k_bf)
                ):
                    ps = psum.tile([P, CH], F32, name=f"{nm}_ps", tag="mm", bufs=2)
                    for ct in range(CT):
                        nc.tensor.matmul(
                            ps,
                            lhsT=w_sb[wname][:, ct, ot * P:(ot + 1) * P],
                            rhs=r_bf[:, ct, nsl],
                            start=(ct == 0), stop=(ct == CT - 1),
                        )
                    nc.scalar.activation(
                        dst[:, ot, nsl], ps, AF.Identity,
                        bias=cols[:, cidx, ot:ot + 1],
                    )
        for nt in range(NT):
            ps = psum.tile([P, CH], F32, name="v_ps", tag="mm", bufs=2)
            for ct in range(CT):
                nc.tensor.matmul(
                    ps,
                    lhsT=r_bf[:, ct, nt * P:(nt + 1) * P],
                    rhs=w_sb["wv"][:, ct, :],
                    start=(ct == 0), stop=(ct == CT - 1),
                )
            nc.scalar.copy(vT_bf[:, nt, :], ps)

        # ---------- attention ----------
        outn_bf = act.tile([P, CT, N], BF16, name="outn_bf", tag="outn", bufs=1)
        for chn in range(NCH):
            isl = slice(chn * CH, (chn + 1) * CH)
            att_ps = psum.tile([P, CT, CH], F32, name="att_ps", tag="att", bufs=1)
            sums_ps = psum.tile([P, CH], F32, name="sums_ps", tag="mm", bufs=2)
            if ATT_FP8:
                for jt2 in range(NT // 2):
                    e_f8 = small.tile([P, 2, CH], FP8, name="e_f8", tag="E", bufs=3)
                    for h in range(2):
                        jt = 2 * jt2 + h
                        s_ps = psum.tile([P, CH], F32, name="s_ps", tag="scores", bufs=2)
                        for a in range(CT // 2):
                            nc.tensor.matmul(
                                s_ps,
                                lhsT=k_bf[:, 2 * a:2 * a + 2, jt * P:(jt + 1) * P],
                                rhs=q_bf[:, 2 * a:2 * a + 2, isl],
                                start=(a == 0), stop=(a == CT // 2 - 1),
                                perf_mode=DR,
                            )
                        nc.scalar.activation(e_f8[:, h, :], s_ps, AF.Exp, scale=SCALE)
                    for ct in range(CT):
                        nc.tensor.matmul(
                            att_ps[:, ct, :],
                            lhsT=vT_bf[:, 2 * jt2:2 * jt2 + 2, ct * P:(ct + 1) * P],
                            rhs=e_f8,
                            start=(jt2 == 0), stop=(jt2 == NT // 2 - 1),
                            perf_mode=DR,
                        )
                    nc.tensor.matmul(
                        sums_ps, lhsT=ones8, rhs=e_f8,
                        start=(jt2 == 0), stop=(jt2 == NT // 2 - 1),
                        perf_mode=DR,
                    )
            else:
                for jt in range(NT):
                    s_ps = psum.tile([P, CH], F32, name="s_ps", tag="scores", bufs=2)
                    for ct in range(CT):
                        nc.tensor.matmul(
                            s_ps,
                            lhsT=k_bf[:, ct, jt * P:(jt + 1) * P],
                            rhs=q_bf[:, ct, isl],
                            start=(ct == 0), stop=(ct == CT - 1),
                        )
                    e_bf = small.tile([P, CH], BF16, name="e_bf", tag="E", bufs=4)
                    nc.scalar.activation(e_bf, s_ps, AF.Exp, scale=SCALE)
                    for ct in range(CT):
                        nc.tensor.matmul(
                            att_ps[:, ct, :],
                            lhsT=vT_bf[:, jt, ct * P:(ct + 1) * P],
                            rhs=e_bf,
                            start=(jt == 0), stop=(jt == NT - 1),
                        )
                    nc.tensor.matmul(
                        sums_ps, lhsT=onesm, rhs=e_bf,
                        start=(jt == 0), stop=(jt == NT - 1),
                    )
            recip = small.tile([P, CH], F32, name="recip", tag="recip", bufs=2)
            nc.vector.reciprocal(recip, sums_ps)
            for ct in range(CT):
                nc.vector.tensor_mul(
                    outn_bf[:, ct, isl], att_ps[:, ct, :], recip
                )

        # ---------- proj + residual ----------
        y_t = act.tile([P, CT, N], F32, name="y_t", tag="y", bufs=2)
        for ot in range(CT):
            for chn in range(NCH):
                nsl = slice(chn * CH, (chn + 1) * CH)
                ps = psum.tile([P, CH], F32, name="p_ps", tag="mm", bufs=2)
                for ct in range(CT):
                    nc.tensor.matmul(
                        ps,
                        lhsT=w_sb["wp"][:, ct, ot * P:(ot + 1) * P],
                        rhs=outn_bf[:, ct, nsl],
                        start=(ct == 0), stop=(ct == CT - 1),
                    )
                pr = small.tile([P, CH], F32, name="pr", tag="pr", bufs=3)
                nc.scalar.activation(
                    pr, ps, AF.Identity, bias=cols[:, 4, ot:ot + 1]
                )
                nc.vector.tensor_add(
                    y_t[:, ot, nsl], pr, x_t[:, ot, nsl]
                )
        nc.sync.dma_start(
            out=y_out[b].rearrange("(t p) n -> p t n", p=P), in_=y_t
        )


def _prep_in_maps(inputs) -> list[dict]:
    f32 = np.float32
    x = np.asarray(inputs["x"], f32).reshape(B, C, N)

    def t_bf(w):
        return np.ascontiguousarray(np.asarray(w, f32).T).astype(ml_dtypes.bfloat16)

    def packc(v):
        return np.ascontiguousarray(np.asarray(v, f32).reshape(CT, P).T)

    pb = (
        np.asarray(inputs["wp"], f32) @ np.asarray(inputs["bv"], f32)
        + np.asarray(inputs["bp"], f32)
    )
    cols = np.ascontiguousarray(
        np.stack(
            [
                packc(inputs["gn_w"]), packc(inputs["gn_b"]),
                packc(inputs["bq"]), packc(inputs["bk"]), packc(pb),
            ],
            axis=1,
        )
    )  # [P, 5, CT]
    mmat = np.kron(
        np.eye(P // GS, dtype=f32), np.ones((GS, GS), f32)
    )  # [128,128] block-diagonal group-sum matrix
    onesm = np.ones((P, P), ml_dtypes.bfloat16)
    shared = dict(
        wqT=t_bf(inputs["wq"]), wkT=t_bf(inputs["wk"]),
        wvT=t_bf(inputs["wv"]), wpT=t_bf(inputs["wp"]),
        cols=cols, mmat=mmat, onesm=onesm,
    )
    return [
        dict(x_in=np.ascontiguousarray(x[c * BPC:(c + 1) * BPC]), **shared)
        for c in range(NCORES)
    ]


_PROG = None


def _run(inputs, **spmd_kwargs):
    global _PROG
    if _PROG is None:
        _PROG = _build_program()
    in_maps = _prep_in_maps(inputs)
    res = run_bass_kernel_spmd(_PROG, in_maps, list(range(NCORES)), **spmd_kwargs)
    y = np.concatenate(
        [np.asarray(res.results[i]["y_out"], np.float32) for i in range(NCORES)],
        axis=0,
    ).reshape(B, C, HH, WW)
    return y, res


def kernel(**inputs) -> np.ndarray:
    y, _ = _run(inputs)
    return y


# revision 23
# speedup vs baseline: 15.1616x; 1.0665x over previous
"""AttnBlock (GroupNorm + spatial self-attention + residual) on 8 TRN2 NeuronCores.

Data-parallel over batch: B=16 -> 2 batch elements per core. Each core runs the
full attention block for its 2 batch elements entirely on-chip; no collectives.

Layouts (per batch element, per core):
  x, r      : [c-part, n]   channel on partitions (4 tiles of 128), spatial free
  q, k      : [c-part, n]   from matmul(lhsT=wT, rhs=r)
  vT        : [n-part, c]   from matmul(lhsT=r, rhs=wT)  (transposed projection)
  scoresT   : [j-part, i]   = k^T q per (j-tile, i-chunk), accumulated over c
  E=exp(s)  : [j-part, i]   bf16, softmax numerator (logits are tiny: no max-sub)
  sums      : all-ones[128,128] @ E accumulates Sum_j E -> broadcast over partitions
  out       : [c-part, i]   = vT^T @ E accumulated over j-tiles, then * 1/sums
  proj, y   : [c-part, n]

Bias folding: bq/bk added at PSUM evac (per-partition bias). bv/bp folded on the
host into pb = wp@bv + bp (valid since sum_j softmax = 1), added at proj evac.
"""

import numpy as np
import ml_dtypes

import concourse.bass as bass
import concourse.bacc as bacc
import concourse.mybir as mybir
import concourse.tile as tile
from concourse.bass_utils import run_bass_kernel_spmd

B, C, HH, WW = 16, 512, 32, 32
N = HH * WW            # 1024 spatial positions
G = 32                 # groupnorm groups
GS = C // G            # 16 channels per group
EPS = 1e-6
P = 128
CT = C // P            # 4 channel tiles
NT = N // P            # 8 spatial tiles
CH = 512               # free-dim chunk (one PSUM bank of fp32)
NCH = N // CH          # 2 chunks
NCORES = 8
BPC = B // NCORES      # 2 batch elements per core
SCALE = float(int(C) ** -0.5)

F32 = mybir.dt.float32
BF16 = mybir.dt.bfloat16
FP8 = mybir.dt.float8e4
AF = mybir.ActivationFunctionType
ATT_FP8 = True          # fp8e4m3 + DoubleRow for scores/out/sums matmuls
DR = mybir.MatmulPerfMode.DoubleRow


def _build_program(loop_reps: int = 1) -> bass.Bass:
    nc = bacc.Bacc("TRN2", target_bir_lowering=False, num_devices=NCORES)

    x_in = nc.declare_dram_parameter("x_in", [BPC, C, N], F32, isOutput=False)
    w_in = {
        w: nc.declare_dram_parameter(w + "T", [C, C], BF16, isOutput=False)
        for w in ("wq", "wk", "wv", "wp")
    }
    # cols[:, 0]=gn_w, 1=gn_b, 2=bq, 3=bk, 4=pb   (per-partition packing, [P, 5, CT])
    cols_in = nc.declare_dram_parameter("cols", [P, 5, CT], F32, isOutput=False)
    mmat_in = nc.declare_dram_parameter("mmat", [P, P], F32, isOutput=False)
    ones_in = nc.declare_dram_parameter("onesm", [P, P], BF16, isOutput=False)
    y_out = nc.declare_dram_parameter("y_out", [BPC, C, N], F32, isOutput=True)

    with tile.TileContext(nc) as tc:
        with (
            tc.tile_pool(name="const", bufs=1) as const,
            tc.tile_pool(name="act", bufs=1) as act,
            tc.tile_pool(name="small", bufs=2) as small,
            tc.tile_pool(name="psum", bufs=1, space="PSUM") as psum,
        ):
            cols = const.tile([P, 5, CT], F32, name="cols_sb", tag="cols_sb")
            nc.gpsimd.dma_start(out=cols, in_=cols_in[:, :, :])
            mmat = const.tile([P, P], F32, name="mmat_sb", tag="mmat_sb")
            nc.gpsimd.dma_start(out=mmat, in_=mmat_in[:, :])
            onesm = const.tile([P, P], BF16, name="ones_sb", tag="ones_sb")
            nc.gpsimd.dma_start(out=onesm, in_=ones_in[:, :])
            ones8 = const.tile([P, 2, P], FP8, name="ones8_sb", tag="ones8_sb")
            nc.gpsimd.memset(ones8, 1.0)
            eps_sb = const.tile([P, 1], F32, name="eps_sb", tag="eps_sb")
            nc.vector.memset(eps_sb, EPS)
            w_sb = {}
            for w in ("wq", "wk", "wv", "wp"):
                wt = const.tile([P, CT, C], BF16, name=f"{w}_sb", tag=f"{w}_sb")
                nc.scalar.dma_start(out=wt, in_=w_in[w].rearrange("(t p) o -> p t o", p=P))
                w_sb[w] = wt

            import contextlib
            loop_cm = (
                tc.For_i(0, loop_reps, 1) if loop_reps > 1
                else contextlib.nullcontext()
            )
            with loop_cm:
                _emit_body(nc, tc, act, small, psum, x_in, y_out, w_sb, cols,
                           mmat, onesm, ones8, eps_sb)
    nc.compile()
    return nc


def _emit_body(nc, tc, act, small, psum, x_in, y_out, w_sb, cols, mmat, onesm,
               ones8, eps_sb):
    xs, rs = [], []
    # ---------- Phase 1: GroupNorm for both batches (per-c-tile pipeline) ----
    # Hoisted ahead of all projections so DVE/ACT compute batch b+1's GN while
    # PE runs batch b's matmuls, and so PE work starts after only one c-tile
    # of x has landed.
    for b in range(BPC):
        x_t = act.tile([P, CT, N], F32, name="x_t", tag="x", bufs=2)
        r_bf = act.tile([P, CT, N], BF16, name="r_bf", tag="r", bufs=2)
        bn6 = small.tile([P, CT, 2, 6], F32, name="bn6", tag="bn6")
        mv = small.tile([P, CT, 2], F32, name="mv", tag="mv")
        msq = small.tile([P, CT], F32, name="msq", tag="msq")
        gsc = small.tile([P, CT, 2], F32, name="gsc", tag="gsc")
        var_t = small.tile([P, CT], F32, name="var_t", tag="var_t")
        scl = small.tile([P, CT], F32, name="scl", tag="scl")
        sh_t = small.tile([P, CT], F32, name="sh_t", tag="sh_t")
        xr = x_in[b].rearrange("(t p) n -> p t n", p=P)
        dma_engs = (nc.sync, nc.sync, nc.sync, nc.sync)
        for ct in range(CT):
            dma_engs[ct % 4].dma_start(out=x_t[:, ct, :], in_=xr[:, ct, :])
            for h in range(2):
                nc.vector.bn_stats(
                    out=bn6[:, ct, h, :], in_=x_t[:, ct, h * CH:(h + 1) * CH]
                )
            nc.vector.bn_aggr(out=mv[:, ct, :], in_=bn6[:, ct, :, :])
            # (mean, var) -> (mean, E[x^2]) in place
            nc.vector.tensor_mul(msq[:, ct:ct + 1], mv[:, ct, 0:1], mv[:, ct, 0:1])
            nc.vector.tensor_add(mv[:, ct, 1:2], mv[:, ct, 1:2], msq[:, ct:ct + 1])
            # group-sum across 16-channel blocks (block-diagonal matmul)
            gn_ps = psum.tile([P, 2], F32, name="gn_ps", tag="mm", bufs=2)
            nc.tensor.matmul(gn_ps, lhsT=mmat, rhs=mv[:, ct, :],
                             start=True, stop=True)
            nc.scalar.mul(gsc[:, ct, :], gn_ps, 1.0 / GS)
            nc.vector.tensor_mul(msq[:, ct:ct + 1], gsc[:, ct, 0:1], gsc[:, ct, 0:1])
            nc.vector.tensor_sub(var_t[:, ct:ct + 1], gsc[:, ct, 1:2], msq[:, ct:ct + 1])
            nc.scalar.activation(var_t[:, ct:ct + 1], var_t[:, ct:ct + 1],
                                 AF.Sqrt, bias=eps_sb)
            nc.vector.reciprocal(var_t[:, ct:ct + 1], var_t[:, ct:ct + 1])
            nc.vector.tensor_mul(scl[:, ct:ct + 1], cols[:, 0, ct:ct + 1],
                                 var_t[:, ct:ct + 1])
            nc.vector.tensor_mul(sh_t[:, ct:ct + 1], gsc[:, ct, 0:1],
                                 scl[:, ct:ct + 1])
            nc.vector.tensor_sub(sh_t[:, ct:ct + 1], cols[:, 1, ct:ct + 1],
                                 sh_t[:, ct:ct + 1])
            nc.scalar.activation(
                r_bf[:, ct, :], x_t[:, ct, :], AF.Identity,
                scale=scl[:, ct:ct + 1], bias=sh_t[:, ct:ct + 1],
            )
        xs.append(x_t)
        rs.append(r_bf)

    # ---------- Phase 2: q/k/v for both batches ----------
    qs, ks, vs = [], [], []
    for b in range(BPC):
        r_bf = rs[b]
        ATT_DT = FP8 if ATT_FP8 else BF16
        q_bf = act.tile([P, CT, N], ATT_DT, name="q_bf", tag="q", bufs=2)
        k_bf = act.tile([P, CT, N], ATT_DT, name="k_bf", tag="k", bufs=2)
        vT_bf = act.tile([P, NT, C], ATT_DT, name="vT_bf", tag="v", bufs=2)
        grp = 0
        for wname, cidx, dst in (("wk", 3, k_bf), ("wq", 2, q_bf)):
            for chn in range(NCH):
                nsl = slice(chn * CH, (chn + 1) * CH)
                for ot in range(CT):
                    ps = psum.tile([P, CH], F32, name="qk_ps",
                                   tag=("mm" if grp % 2 else "scores"), bufs=2)
                    grp += 1
                    for ct in range(CT):
                        nc.tensor.matmul(
                            ps,
                            lhsT=w_sb[wname][:, ct, ot * P:(ot + 1) * P],
                            rhs=r_bf[:, ct, nsl],
                            start=(ct == 0), stop=(ct == CT - 1),
                        )
                    nc.scalar.activation(
                        dst[:, ot, nsl], ps, AF.Identity,
                        bias=cols[:, cidx, ot:ot + 1],
                    )
        for nt in range(NT):
            ps = psum.tile([P, CH], F32, name="v_ps",
                           tag=("mm" if grp % 2 else "scores"), bufs=2)
            grp += 1
            for ct in range(CT):
                nc.tensor.matmul(
                    ps,
                    lhsT=r_bf[:, ct, nt * P:(nt + 1) * P],
                    rhs=w_sb["wv"][:, ct, :],
                    start=(ct == 0), stop=(ct == CT - 1),
                )
            nc.vector.tensor_copy(vT_bf[:, nt, :], ps)
        qs.append(q_bf)
        ks.append(k_bf)
        vs.append(vT_bf)

    # ---------- Phase 3: per-batch, per-chunk attention + proj ----------
    for b in range(BPC):
        x_t, q_bf, k_bf, vT_bf = xs[b], qs[b], ks[b], vs[b]
        outn_bf = act.tile([P, CT, N], BF16, name="outn_bf", tag="outn", bufs=2)
        y_t = act.tile([P, CT, N], F32, name="y_t", tag="y", bufs=2)
        for chn in range(NCH):
            isl = slice(chn * CH, (chn + 1) * CH)
            att_ps = psum.tile([P, CT, CH], F32, name="att_ps", tag="att", bufs=1)
            sums_ps = psum.tile([P, CH], F32, name="sums_ps", tag="mm", bufs=2)
            if ATT_FP8:
                for jt2 in range(NT // 2):
                    e_f8 = small.tile([P, 2, CH], FP8, name="e_f8", tag="E", bufs=3)
                    for h in range(2):
                        jt = 2 * jt2 + h
                        s_ps = psum.tile([P, CH], F32, name="s_ps", tag="scores", bufs=2)
                        for a in range(CT // 2):
                            nc.tensor.matmul(
                                s_ps,
                                lhsT=k_bf[:, 2 * a:2 * a + 2, jt * P:(jt + 1) * P],
                                rhs=q_bf[:, 2 * a:2 * a + 2, isl],
                                start=(a == 0), stop=(a == CT // 2 - 1),
                                perf_mode=DR,
                            )
                        nc.scalar.activation(e_f8[:, h, :], s_ps, AF.Exp, scale=SCALE)
                    for ct in range(CT):
                        nc.tensor.matmul(
                            att_ps[:, ct, :],
                            lhsT=vT_bf[:, 2 * jt2:2 * jt2 + 2, ct * P:(ct + 1) * P],
                            rhs=e_f8,
                            start=(jt2 == 0), stop=(jt2 == NT // 2 - 1),
                            perf_mode=DR,
                        )
                    nc.tensor.matmul(
                        sums_ps, lhsT=ones8, rhs=e_f8,
                        start=(jt2 == 0), stop=(jt2 == NT // 2 - 1),
                        perf_mode=DR,
                    )
            else:
                for jt in range(NT):
                    s_ps = psum.tile([P, CH], F32, name="s_ps", tag="scores", bufs=2)
                    for ct in range(CT):
                        nc.tensor.matmul(
                            s_ps,
                            lhsT=k_bf[:, ct, jt * P:(jt + 1) * P],
                            rhs=q_bf[:, ct, isl],
                            start=(ct == 0), stop=(ct == CT - 1),
                        )
                    e_bf = small.tile([P, CH], BF16, name="e_bf", tag="E", bufs=4)
                    nc.scalar.activation(e_bf, s_ps, AF.Exp, scale=SCALE)
                    for ct in range(CT):
                        nc.tensor.matmul(
                            att_ps[:, ct, :],
                            lhsT=vT_bf[:, jt, ct * P:(ct + 1) * P],
                            rhs=e_bf,
                            start=(jt == 0), stop=(jt == NT - 1),
                        )
                    nc.tensor.matmul(
                        sums_ps, lhsT=onesm, rhs=e_bf,
                        start=(jt == 0), stop=(jt == NT - 1),
                    )
            recip = small.tile([P, CH], F32, name="recip", tag="recip", bufs=2)
            nc.vector.reciprocal(recip, sums_ps)
            for ct in range(CT):
                nc.vector.tensor_mul(
                    outn_bf[:, ct, isl], att_ps[:, ct, :], recip
                )
        for ot in range(CT):
            for chn in range(NCH):
                nsl = slice(chn * CH, (chn + 1) * CH)
                ps = psum.tile([P, CH], F32, name="p_ps",
                               tag=("mm" if (ot + chn) % 2 else "scores"), bufs=2)
                for ct in range(CT):
                    nc.tensor.matmul(
                        ps,
                        lhsT=w_sb["wp"][:, ct, ot * P:(ot + 1) * P],
                        rhs=outn_bf[:, ct, nsl],
                        start=(ct == 0), stop=(ct == CT - 1),
                    )
                nc.vector.scalar_tensor_tensor(
                    out=y_t[:, ot, nsl], in0=ps, scalar=cols[:, 4, ot:ot + 1],
                    in1=x_t[:, ot, nsl],
                    op0=mybir.AluOpType.add, op1=mybir.AluOpType.add,
                )
        nc.sync.dma_start(
            out=y_out[b].rearrange("(t p) n -> p t n", p=P), in_=y_t
        )


def _prep_in_maps(inputs) -> list[dict]:
    f32 = np.float32
    x = np.asarray(inputs["x"], f32).reshape(B, C, N)

    def t_bf(w):
        return np.ascontiguousarray(np.asarray(w, f32).T).astype(ml_dtypes.bfloat16)

    def packc(v):
        return np.ascontiguousarray(np.asarray(v, f32).reshape(CT, P).T)

    pb = (
        np.asarray(inputs["wp"], f32) @ np.asarray(inputs["bv"], f32)
        + np.asarray(inputs["bp"], f32)
    )
    cols = np.ascontiguousarray(
        np.stack(
            [
                packc(inputs["gn_w"]), packc(inputs["gn_b"]),
                packc(inputs["bq"]), packc(inputs["bk"]), packc(pb),
            ],
            axis=1,
        )
    )  # [P, 5, CT]
    mmat = np.kron(
        np.eye(P // GS, dtype=f32), np.ones((GS, GS), f32)
    )  # [128,128] block-diagonal group-sum matrix
    onesm = np.ones((P, P), ml_dtypes.bfloat16)
    shared = dict(
        wqT=t_bf(inputs["wq"]), wkT=t_bf(inputs["wk"]),
        wvT=t_bf(inputs["wv"]), wpT=t_bf(inputs["wp"]),
        cols=cols, mmat=mmat, onesm=onesm,
    )
    return [
        dict(x_in=np.ascontiguousarray(x[c * BPC:(c + 1) * BPC]), **shared)
        for c in range(NCORES)
    ]


_PROG = None


def _run(inputs, **spmd_kwargs):
    global _PROG
    if _PROG is None:
        _PROG = _build_program()
    in_maps = _prep_in_maps(inputs)
    res = run_bass_kernel_spmd(_PROG, in_maps, list(range(NCORES)), **spmd_kwargs)
    y = np.concatenate(
        [np.asarray(res.results[i]["y_out"], np.float32) for i in range(NCORES)],
        axis=0,
    ).reshape(B, C, HH, WW)
    return y, res


def kernel(**inputs) -> np.ndarray:
    y, _ = _run(inputs)
    return y


# revision 25
# speedup vs baseline: 18.0372x; 1.1897x over previous
"""AttnBlock (GroupNorm + spatial self-attention + residual) on 8 TRN2 NeuronCores.

Data-parallel over batch: B=16 -> 2 batch elements per core. Each core runs the
full attention block for its 2 batch elements entirely on-chip; no collectives.

Layouts (per batch element, per core):
  x, r      : [c-part, n]   channel on partitions (4 tiles of 128), spatial free
  q, k      : [c-part, n]   from matmul(lhsT=wT, rhs=r)
  vT        : [n-part, c]   from matmul(lhsT=r, rhs=wT)  (transposed projection)
  scoresT   : [j-part, i]   = k^T q per (j-tile, i-chunk), accumulated over c
  E=exp(s)  : [j-part, i]   bf16, softmax numerator (logits are tiny: no max-sub)
  sums      : all-ones[128,128] @ E accumulates Sum_j E -> broadcast over partitions
  out       : [c-part, i]   = vT^T @ E accumulated over j-tiles, then * 1/sums
  proj, y   : [c-part, n]

Bias folding: bq/bk added at PSUM evac (per-partition bias). bv/bp folded on the
host into pb = wp@bv + bp (valid since sum_j softmax = 1), added at proj evac.
"""

import numpy as np
import ml_dtypes

import concourse.bass as bass
import concourse.bacc as bacc
import concourse.mybir as mybir
import concourse.tile as tile
from concourse.bass_utils import run_bass_kernel_spmd

B, C, HH, WW = 16, 512, 32, 32
N = HH * WW            # 1024 spatial positions
G = 32                 # groupnorm groups
GS = C // G            # 16 channels per group
EPS = 1e-6
P = 128
CT = C // P            # 4 channel tiles
NT = N // P            # 8 spatial tiles
CH = 512               # free-dim chunk (one PSUM bank of fp32)
NCH = N // CH          # 2 chunks
NCORES = 8
BPC = B // NCORES      # 2 batch elements per core
SCALE = float(int(C) ** -0.5)

F32 = mybir.dt.float32
BF16 = mybir.dt.bfloat16
FP8 = mybir.dt.float8e4
AF = mybir.ActivationFunctionType
ATT_FP8 = True          # fp8e4m3 + DoubleRow for scores/out/sums matmuls
DR = mybir.MatmulPerfMode.DoubleRow


def _build_program(loop_reps: int = 1) -> bass.Bass:
    nc = bacc.Bacc("TRN2", target_bir_lowering=False, num_devices=NCORES)

    x_in = nc.declare_dram_parameter("x_in", [BPC, C, N], F32, isOutput=False)
    w_in = {
        w: nc.declare_dram_parameter(
            w + "T", [C, C], FP8 if (ATT_FP8 and w != "wp") else BF16,
            isOutput=False)
        for w in ("wq", "wk", "wv", "wp")
    }
    # cols[:, 0]=gn_w, 1=gn_b, 2=bq, 3=bk, 4=pb   (per-partition packing, [P, 5, CT])
    cols_in = nc.declare_dram_parameter("cols", [P, 5, CT], F32, isOutput=False)
    mmat_in = nc.declare_dram_parameter("mmat", [P, P], F32, isOutput=False)
    ones_in = nc.declare_dram_parameter("onesm", [P, P], BF16, isOutput=False)
    y_out = nc.declare_dram_parameter("y_out", [BPC, C, N], F32, isOutput=True)

    with tile.TileContext(nc) as tc:
        with (
            tc.tile_pool(name="const", bufs=1) as const,
            tc.tile_pool(name="act", bufs=1) as act,
            tc.tile_pool(name="small", bufs=2) as small,
            tc.tile_pool(name="psum", bufs=1, space="PSUM") as psum,
        ):
            cols = const.tile([P, 5, CT], F32, name="cols_sb", tag="cols_sb")
            nc.gpsimd.dma_start(out=cols, in_=cols_in[:, :, :])
            mmat = const.tile([P, P], F32, name="mmat_sb", tag="mmat_sb")
            nc.gpsimd.dma_start(out=mmat, in_=mmat_in[:, :])
            onesm = const.tile([P, P], BF16, name="ones_sb", tag="ones_sb")
            nc.gpsimd.dma_start(out=onesm, in_=ones_in[:, :])
            ones8 = const.tile([P, 2, P], FP8, name="ones8_sb", tag="ones8_sb")
            nc.gpsimd.memset(ones8, 1.0)
            eps_sb = const.tile([P, 1], F32, name="eps_sb", tag="eps_sb")
            nc.vector.memset(eps_sb, EPS)
            w_sb = {}
            for w in ("wq", "wk", "wv", "wp"):
                wdt = FP8 if (ATT_FP8 and w != "wp") else BF16
                wt = const.tile([P, CT, C], wdt, name=f"{w}_sb", tag=f"{w}_sb")
                nc.scalar.dma_start(out=wt, in_=w_in[w].rearrange("(t p) o -> p t o", p=P))
                w_sb[w] = wt

            import contextlib
            loop_cm = (
                tc.For_i(0, loop_reps, 1) if loop_reps > 1
                else contextlib.nullcontext()
            )
            with loop_cm:
                _emit_body(nc, tc, act, small, psum, x_in, y_out, w_sb, cols,
                           mmat, onesm, ones8, eps_sb)
    nc.compile()
    return nc


def _emit_body(nc, tc, act, small, psum, x_in, y_out, w_sb, cols, mmat, onesm,
               ones8, eps_sb):
    xs, rs = [], []
    # ---------- Phase 1: GroupNorm for both batches (per-c-tile pipeline) ----
    # Hoisted ahead of all projections so DVE/ACT compute batch b+1's GN while
    # PE runs batch b's matmuls, and so PE work starts after only one c-tile
    # of x has landed.
    for b in range(BPC):
        x_t = act.tile([P, CT, N], F32, name="x_t", tag="x", bufs=2)
        r_bf = act.tile([P, CT, N], FP8 if ATT_FP8 else BF16, name="r_bf",
                        tag="r", bufs=2)
        bn6 = small.tile([P, CT, 2, 6], F32, name="bn6", tag="bn6")
        mv = small.tile([P, CT, 2], F32, name="mv", tag="mv")
        msq = small.tile([P, CT], F32, name="msq", tag="msq")
        gsc = small.tile([P, CT, 2], F32, name="gsc", tag="gsc")
        var_t = small.tile([P, CT], F32, name="var_t", tag="var_t")
        scl = small.tile([P, CT], F32, name="scl", tag="scl")
        sh_t = small.tile([P, CT], F32, name="sh_t", tag="sh_t")
        xr = x_in[b].rearrange("(t p) n -> p t n", p=P)
        dma_engs = (nc.sync, nc.sync, nc.sync, nc.sync)
        for ct in range(CT):
            dma_engs[ct % 4].dma_start(out=x_t[:, ct, :], in_=xr[:, ct, :])
            for h in range(2):
                nc.vector.bn_stats(
                    out=bn6[:, ct, h, :], in_=x_t[:, ct, h * CH:(h + 1) * CH]
                )
            nc.vector.bn_aggr(out=mv[:, ct, :], in_=bn6[:, ct, :, :])
            # (mean, var) -> (mean, E[x^2]) in place
            nc.vector.tensor_mul(msq[:, ct:ct + 1], mv[:, ct, 0:1], mv[:, ct, 0:1])
            nc.vector.tensor_add(mv[:, ct, 1:2], mv[:, ct, 1:2], msq[:, ct:ct + 1])
            # group-sum across 16-channel blocks (block-diagonal matmul)
            gn_ps = psum.tile([P, 2], F32, name="gn_ps", tag="mm", bufs=2)
            nc.tensor.matmul(gn_ps, lhsT=mmat, rhs=mv[:, ct, :],
                             start=True, stop=True)
            nc.scalar.mul(gsc[:, ct, :], gn_ps, 1.0 / GS)
            nc.vector.tensor_mul(msq[:, ct:ct + 1], gsc[:, ct, 0:1], gsc[:, ct, 0:1])
            nc.vector.tensor_sub(var_t[:, ct:ct + 1], gsc[:, ct, 1:2], msq[:, ct:ct + 1])
            nc.scalar.activation(var_t[:, ct:ct + 1], var_t[:, ct:ct + 1],
                                 AF.Sqrt, bias=eps_sb)
            nc.vector.reciprocal(var_t[:, ct:ct + 1], var_t[:, ct:ct + 1])
            nc.vector.tensor_mul(scl[:, ct:ct + 1], cols[:, 0, ct:ct + 1],
                                 var_t[:, ct:ct + 1])
            nc.vector.tensor_mul(sh_t[:, ct:ct + 1], gsc[:, ct, 0:1],
                                 scl[:, ct:ct + 1])
            nc.vector.tensor_sub(sh_t[:, ct:ct + 1], cols[:, 1, ct:ct + 1],
                                 sh_t[:, ct:ct + 1])
            nc.scalar.activation(
                r_bf[:, ct, :], x_t[:, ct, :], AF.Identity,
                scale=scl[:, ct:ct + 1], bias=sh_t[:, ct:ct + 1],
            )
        xs.append(x_t)
        rs.append(r_bf)

    # ---------- Phase 2: q/k/v for both batches ----------
    qs, ks, vs = [], [], []
    for b in range(BPC):
        r_bf = rs[b]
        ATT_DT = FP8 if ATT_FP8 else BF16
        q_bf = act.tile([P, CT, N], ATT_DT, name="q_bf", tag="q", bufs=2)
        k_bf = act.tile([P, CT, N], ATT_DT, name="k_bf", tag="k", bufs=2)
        vT_bf = act.tile([P, NT, C], ATT_DT, name="vT_bf", tag="v", bufs=2)
        grp = 0
        for wname, cidx, dst in (("wk", 3, k_bf), ("wq", 2, q_bf)):
            for chn in range(NCH):
                nsl = slice(chn * CH, (chn + 1) * CH)
                for ot in range(CT):
                    ps = psum.tile([P, CH], F32, name="qk_ps",
                                   tag=("mm" if grp % 2 else "scores"), bufs=2)
                    grp += 1
                    if ATT_FP8:
                        for a in range(CT // 2):
                            nc.tensor.matmul(
                                ps,
                                lhsT=w_sb[wname][:, 2 * a:2 * a + 2,
                                                 ot * P:(ot + 1) * P],
                                rhs=r_bf[:, 2 * a:2 * a + 2, nsl],
                                start=(a == 0), stop=(a == CT // 2 - 1),
                                perf_mode=DR,
                            )
                    else:
                        for ct in range(CT):
                            nc.tensor.matmul(
                                ps,
                                lhsT=w_sb[wname][:, ct, ot * P:(ot + 1) * P],
                                rhs=r_bf[:, ct, nsl],
                                start=(ct == 0), stop=(ct == CT - 1),
                            )
                    nc.scalar.activation(
                        dst[:, ot, nsl], ps, AF.Identity,
                        bias=cols[:, cidx, ot:ot + 1],
                    )
        for nt in range(NT):
            ps = psum.tile([P, CH], F32, name="v_ps",
                           tag=("mm" if grp % 2 else "scores"), bufs=2)
            grp += 1
            if ATT_FP8:
                for a in range(CT // 2):
                    nc.tensor.matmul(
                        ps,
                        lhsT=r_bf[:, 2 * a:2 * a + 2, nt * P:(nt + 1) * P],
                        rhs=w_sb["wv"][:, 2 * a:2 * a + 2, :],
                        start=(a == 0), stop=(a == CT // 2 - 1),
                        perf_mode=DR,
                    )
            else:
                for ct in range(CT):
                    nc.tensor.matmul(
                        ps,
                        lhsT=r_bf[:, ct, nt * P:(nt + 1) * P],
                        rhs=w_sb["wv"][:, ct, :],
                        start=(ct == 0), stop=(ct == CT - 1),
                    )
            nc.vector.tensor_copy(vT_bf[:, nt, :], ps)
        qs.append(q_bf)
        ks.append(k_bf)
        vs.append(vT_bf)

    # ---------- Phase 3: per-batch, per-chunk attention + proj ----------
    for b in range(BPC):
        x_t, q_bf, k_bf, vT_bf = xs[b], qs[b], ks[b], vs[b]
        outn_bf = act.tile([P, CT, N], BF16, name="outn_bf", tag="outn", bufs=2)
        y_t = act.tile([P, CT, N], F32, name="y_t", tag="y", bufs=2)
        for chn in range(NCH):
            isl = slice(chn * CH, (chn + 1) * CH)
            att_ps = psum.tile([P, CT, CH], F32, name="att_ps", tag="att", bufs=1)
            sums_ps = psum.tile([P, CH], F32, name="sums_ps", tag="mm", bufs=2)
            if ATT_FP8:
                for jt2 in range(NT // 2):
                    e_f8 = small.tile([P, 2, CH], FP8, name="e_f8", tag="E", bufs=3)
                    for h in range(2):
                        jt = 2 * jt2 + h
                        s_ps = psum.tile([P, CH], F32, name="s_ps", tag="scores", bufs=2)
                        for a in range(CT // 2):
                            nc.tensor.matmul(
                                s_ps,
                                lhsT=k_bf[:, 2 * a:2 * a + 2, jt * P:(jt + 1) * P],
                                rhs=q_bf[:, 2 * a:2 * a + 2, isl],
                                start=(a == 0), stop=(a == CT // 2 - 1),
                                perf_mode=DR,
                            )
                        nc.scalar.activation(e_f8[:, h, :], s_ps, AF.Exp, scale=SCALE)
                    for ct in range(CT):
                        nc.tensor.matmul(
                            att_ps[:, ct, :],
                            lhsT=vT_bf[:, 2 * jt2:2 * jt2 + 2, ct * P:(ct + 1) * P],
                            rhs=e_f8,
                            start=(jt2 == 0), stop=(jt2 == NT // 2 - 1),
                            perf_mode=DR,
                        )
                    nc.tensor.matmul(
                        sums_ps, lhsT=ones8, rhs=e_f8,
                        start=(jt2 == 0), stop=(jt2 == NT // 2 - 1),
                        perf_mode=DR,
                    )
            else:
                for jt in range(NT):
                    s_ps = psum.tile([P, CH], F32, name="s_ps", tag="scores", bufs=2)
                    for ct in range(CT):
                        nc.tensor.matmul(
                            s_ps,
                            lhsT=k_bf[:, ct, jt * P:(jt + 1) * P],
                            rhs=q_bf[:, ct, isl],
                            start=(ct == 0), stop=(ct == CT - 1),
                        )
                    e_bf = small.tile([P, CH], BF16, name="e_bf", tag="E", bufs=4)
                    nc.scalar.activation(e_bf, s_ps, AF.Exp, scale=SCALE)
                    for ct in range(CT):
                        nc.tensor.matmul(
                            att_ps[:, ct, :],
                            lhsT=vT_bf[:, jt, ct * P:(ct + 1) * P],
                            rhs=e_bf,
                            start=(jt == 0), stop=(jt == NT - 1),
                        )
                    nc.tensor.matmul(
                        sums_ps, lhsT=onesm, rhs=e_bf,
                        start=(jt == 0), stop=(jt == NT - 1),
                    )
            recip = small.tile([P, CH], F32, name="recip", tag="recip", bufs=2)
            nc.vector.reciprocal(recip, sums_ps)
            for ct in range(CT):
                nc.vector.tensor_mul(
                    outn_bf[:, ct, isl], att_ps[:, ct, :], recip
                )
        for ot in range(CT):
            for chn in range(NCH):
                nsl = slice(chn * CH, (chn + 1) * CH)
                ps = psum.tile([P, CH], F32, name="p_ps",
                               tag=("mm" if (ot + chn) % 2 else "scores"), bufs=2)
                for ct in range(CT):
                    nc.tensor.matmul(
                        ps,
                        lhsT=w_sb["wp"][:, ct, ot * P:(ot + 1) * P],
                        rhs=outn_bf[:, ct, nsl],
                        start=(ct == 0), stop=(ct == CT - 1),
                    )
                nc.vector.scalar_tensor_tensor(
                    out=y_t[:, ot, nsl], in0=ps, scalar=cols[:, 4, ot:ot + 1],
                    in1=x_t[:, ot, nsl],
                    op0=mybir.AluOpType.add, op1=mybir.AluOpType.add,
                )
            nc.sync.dma_start(
                out=y_out[b].rearrange("(t p) n -> p t n", p=P)[:, ot, :],
                in_=y_t[:, ot, :],
            )


def _prep_in_maps(inputs) -> list[dict]:
    f32 = np.float32
    x = np.asarray(inputs["x"], f32).reshape(B, C, N)

    def t_bf(w, dt=ml_dtypes.bfloat16):
        return np.ascontiguousarray(np.asarray(w, f32).T).astype(dt)

    def packc(v):
        return np.ascontiguousarray(np.asarray(v, f32).reshape(CT, P).T)

    pb = (
        np.asarray(inputs["wp"], f32) @ np.asarray(inputs["bv"], f32)
        + np.asarray(inputs["bp"], f32)
    )
    cols = np.ascontiguousarray(
        np.stack(
            [
                packc(inputs["gn_w"]), packc(inputs["gn_b"]),
                packc(inputs["bq"]), packc(inputs["bk"]), packc(pb),
            ],
            axis=1,
        )
    )  # [P, 5, CT]
    mmat = np.kron(
        np.eye(P // GS, dtype=f32), np.ones((GS, GS), f32)
    )  # [128,128] block-diagonal group-sum matrix
    onesm = np.ones((P, P), ml_dtypes.bfloat16)
    qkv_dt = ml_dtypes.float8_e4m3 if ATT_FP8 else ml_dtypes.bfloat16
    shared = dict(
        wqT=t_bf(inputs["wq"], qkv_dt), wkT=t_bf(inputs["wk"], qkv_dt),
        wvT=t_bf(inputs["wv"], qkv_dt), wpT=t_bf(inputs["wp"]),
        cols=cols, mmat=mmat, onesm=onesm,
    )
    return [
        dict(x_in=np.ascontiguousarray(x[c * BPC:(c + 1) * BPC]), **shared)
        for c in range(NCORES)
    ]


_PROG = None


def _run(inputs, **spmd_kwargs):
    global _PROG
    if _PROG is None:
        _PROG = _build_program()
    in_maps = _prep_in_maps(inputs)
    res = run_bass_kernel_spmd(_PROG, in_maps, list(range(NCORES)), **spmd_kwargs)
    y = np.concatenate(
        [np.asarray(res.results[i]["y_out"], np.float32) for i in range(NCORES)],
        axis=0,
    ).reshape(B, C, HH, WW)
    return y, res


def kernel(**inputs) -> np.ndarray:
    y, _ = _run(inputs)
    return y
